# revision 21
# baseline (speedup 1.0000x reference)
"""Multi-head attention (B=2, S=4096, D=768, H=12, d_k=64) on 8 TRN2 cores.

Sharding: core c -> batch b = c//4, head group g = c%4 (heads 3g..3g+2).
Each core computes partial = sum_{h in group} softmax(QK^T/8) V @ Wo_h^T
over its batch; host sums the 4 partials per batch and adds bo.

Device kernel (identical SPMD program, per-core data):
  Phase A: QKV projections (fp32r matmuls), Q^T/K^T/V^T produced in
           [head_dim, seq] layout (bf16), V transposed to natural
           [seq, head_dim] layout with a ones column appended (row sums).
  Phase B: per (head, q-chunk of 512): S^T tiles [128k, 512q] via
           64-contraction matmuls (two concurrent row-tiles T0/T8),
           exp on ACT from 2-bank PSUM groups -> bf16, O^T accumulation
           with V|ones (row 64 = softmax sums), per-q normalization via
           reciprocal + gpsimd partition broadcast.
  Phase C: out[qtile] = sum_h O_h^T.T @ Wo_h^T (fp32r), DMA to DRAM.
"""

import numpy as np

import concourse.bass as bass
import concourse.mybir as mybir
import concourse.tile as tile
from concourse import bacc
from concourse.masks import make_identity

F32 = mybir.dt.float32
F32R = mybir.dt.float32r
BF16 = mybir.dt.bfloat16
F16 = mybir.dt.float16
I16 = mybir.dt.int16

N_CORES = 8
B, S, D = 2, 4096, 768
H, DK = 12, 64
HPC = 3            # heads per core
QC = 512           # q-chunk width (free dim of S^T matmuls)
NQC = S // QC      # 8
NKB = S // 128     # 32 k-blocks of 128
XCH = 512          # x streaming chunk (columns of x^T per DMA)
OT_DT = F32R       # dtype of O^T staging

# projection group packing: 5 groups of two 64-dim tensors (by (head, kind))
# kind: 0=Q, 1=K, 2=V
PROJ_GROUPS = [((0, 0), (0, 1)), ((0, 2), (1, 0)), ((1, 1), (1, 2)),
               ((2, 0), (2, 1)), ((2, 2), (2, 2))]


# v3: which of the 16 exp groups (of 2 k-blocks each) per (head, qi) go to
# the DVE via the Schraudolph bit-trick instead of the ACT engine.
DVE_EXP_GROUPS = frozenset({2, 5, 8, 11, 14})


def build_program_v3(repeat=1):
    nc = bacc.Bacc("TRN2", debug=False, num_devices=N_CORES)
    xT_d = nc.dram_tensor("xT", [D, S], F16, kind="ExternalInput").ap()
    wp_d = nc.dram_tensor("wp", [HPC, 2, 6, 128, 128], F16,
                          kind="ExternalInput").ap()
    bp_d = nc.dram_tensor("bp", [128, HPC, 2], F32,
                          kind="ExternalInput").ap()
    wo01_d = nc.dram_tensor("wo01", [128, D], F16, kind="ExternalInput").ap()
    wo2_d = nc.dram_tensor("wo2", [DK, D], F16, kind="ExternalInput").ap()
    out_d = nc.dram_tensor("out", [S, D], F32, kind="ExternalOutput").ap()
    with tile.TileContext(nc) as tc, \
            nc.allow_low_precision("fp16/bf16 attention pipeline"):
        for _ in range(repeat):
            _emit_v3(nc, tc, xT_d, wp_d, bp_d, wo01_d, wo2_d, out_d)
    nc.compile()
    return nc


def _emit_v3(nc, tc, xT_d, wp_d, bp_d, wo01_d, wo2_d, out_d):
    """All-16-bit MHA pipeline for 3 heads of one batch row.

    Layouts (per partition p unless noted):
      x_sb    [128, 6, S] f16  x^T, contraction chunks of 128 (loaded once)
      QKT[j]  [128, S] f16     rows 0:64 = Q^T, 64:128 = K^T
      KQT[j]  [128, S] f16     swapped copy: rows 0:64 = K^T, 64:128 = Q^T
                               (kb-parity trick: even kb contracts on rows
                               0:64, odd kb on rows 64:128 so consecutive
                               LDWEIGHTS/matmuls touch disjoint PE rows)
      V[j]    [128, NKB, 65] bf16  natural layout [dims | ones]
      OT01    [128, S] f16     O^T heads 0 (rows 0:64) and 1 (64:128)
      OT2     [64, S] f16      O^T head 2
    Softmax: es = exp(s/8) in bf16 (ACT exp; DVE Schraudolph groups),
    row sums via the ones column, normalize O^T by 1/sum (recip + gpsimd
    broadcast), out = sum_j O_j^T.T @ Wo_j^T + host-added bo.
    Phase A of head j+1 is interleaved under phase B of head j to keep
    the ACT/DVE exp stream dense.
    """
    import contextlib
    ctx = contextlib.ExitStack()
    with ctx:
        wpool = ctx.enter_context(tc.tile_pool(name="wpool", bufs=1))
        persist = ctx.enter_context(tc.tile_pool(name="persist", bufs=1))
        vtpool = ctx.enter_context(tc.tile_pool(name="vtpool", bufs=2))
        epool = ctx.enter_context(tc.tile_pool(name="epool", bufs=4))
        rpool = ctx.enter_context(tc.tile_pool(name="rpool", bufs=2))
        opool = ctx.enter_context(tc.tile_pool(name="opool", bufs=2))
        ppS = ctx.enter_context(tc.tile_pool(name="ppS", bufs=2, space="PSUM"))
        ppO = ctx.enter_context(tc.tile_pool(name="ppO", bufs=2, space="PSUM"))
        ppA = ctx.enter_context(tc.tile_pool(name="ppA", bufs=2, space="PSUM"))

        # ---- weights ----
        wsb = wpool.tile([128, HPC, 2, 6, 128], F16)
        nc.sync.dma_start(out=wsb,
                          in_=wp_d.rearrange("j g c p m -> p j g c m"))
        bsb = wpool.tile([128, HPC, 2], F32)
        nc.sync.dma_start(out=bsb, in_=bp_d)
        wos01 = wpool.tile([128, D], F16)
        nc.sync.dma_start(out=wos01, in_=wo01_d)
        wos2 = wpool.tile([DK, D], F16)
        nc.sync.dma_start(out=wos2, in_=wo2_d)

        # ---- persistent tensors ----
        x_sb = persist.tile([128, 6, S], F16)
        QKT = [persist.tile([128, S], F16, tag=f"qkt{j}", name=f"qkt{j}")
               for j in range(HPC)]
        KQT = [persist.tile([128, S], F16, tag=f"kqt{j}", name=f"kqt{j}")
               for j in range(HPC)]
        V = [persist.tile([128, NKB, DK + 1], BF16, tag=f"v{j}",
                          name=f"v{j}") for j in range(HPC)]
        OT01 = persist.tile([128, S], F16)
        OT2 = persist.tile([DK, S], F16)

        for j in range(HPC):
            nc.vector.memset(V[j][:, :, DK], 1.0)

        n_xch = S // XCH

        vt01 = persist.tile([128, S], F16)
        vt2 = persist.tile([DK, S], F16)

        def emit_a_chunk(j, ci):
            """Projection chunk ci for head j (plus x DMA for head 0).

            V projections are packed ahead: head 0's chunk computes V0|V1,
            head 1's computes V2, head 2's none.
            """
            cs = slice(ci * XCH, (ci + 1) * XCH)
            if j == 0:
                nc.sync.dma_start(
                    out=x_sb[:, :, cs],
                    in_=xT_d.rearrange("(c p) q -> p c q", p=128)[:, :, cs])
            ps = ppA.tile([128, XCH], F32, tag="ac", name="ps")
            for c in range(6):
                nc.tensor.matmul(
                    ps, lhsT=wsb[:, j, 0, c, :], rhs=x_sb[:, c, cs],
                    start=(c == 0), stop=(c == 5))
            nc.vector.tensor_scalar_add(
                out=QKT[j][:, cs], in0=ps, scalar1=bsb[:, j, 0:1])
            if j == 2:
                return
            ps2 = ppA.tile([128, XCH], F32, tag="ac", name="ps2")
            wide = 128 if j == 0 else 64
            for c in range(6):
                nc.tensor.matmul(
                    ps2[0:wide, :], lhsT=wsb[:, j, 1, c, 0:wide],
                    rhs=x_sb[:, c, cs], start=(c == 0), stop=(c == 5))
            vt = vt01 if j == 0 else vt2
            nc.vector.tensor_scalar_add(
                out=vt[:, cs], in0=ps2[0:wide, :],
                scalar1=bsb[0:wide, j, 1:2])

        def emit_a_tail(j):
            """Q/K swap-dup + V natural layout (V0+V1 at head 0's tail)."""
            nc.sync.dma_start(out=KQT[j][0:64, :], in_=QKT[j][64:128, :])
            nc.sync.dma_start(out=KQT[j][64:128, :], in_=QKT[j][0:64, :])
            srcs = {0: [(0, vt01[0:64, :]), (1, vt01[64:128, :])],
                    1: [(2, vt2[:, :])], 2: []}[j]
            for jv, src in srcs:
                vt16 = vtpool.tile([128, NKB, DK], F16, tag="vt16",
                                   name="vt16", bufs=2)
                nc.sync.dma_start_transpose(out=vt16, in_=src)
                nc.vector.tensor_copy(out=V[jv][:, :, 0:DK], in_=vt16)

        def emit_c(cqi):
            for t in range(cqi * QC // 128, (cqi + 1) * QC // 128):
                ts_ = slice(t * 128, (t + 1) * 128)
                c1 = ppA.tile([128, 512], F32, tag="ac", name="c1")
                c2 = ppA.tile([128, 256], F32, tag="ac", name="c2")
                nc.tensor.matmul(
                    c1, lhsT=OT01[:, ts_], rhs=wos01[:, 0:512],
                    start=True, stop=False)
                nc.tensor.matmul(
                    c1, lhsT=OT2[:, ts_], rhs=wos2[:, 0:512],
                    start=False, stop=True)
                nc.tensor.matmul(
                    c2, lhsT=OT01[:, ts_], rhs=wos01[:, 512:768],
                    start=True, stop=False)
                nc.tensor.matmul(
                    c2, lhsT=OT2[:, ts_], rhs=wos2[:, 512:768],
                    start=False, stop=True)
                ot = opool.tile([128, D], F32, tag="o", name="ot")
                nc.vector.tensor_copy(out=ot[:, 0:512], in_=c1)
                nc.vector.tensor_copy(out=ot[:, 512:768], in_=c2)
                nc.sync.dma_start(out=out_d[ts_, :], in_=ot)

        # phase A for head 0 upfront
        for ci in range(n_xch):
            emit_a_chunk(0, ci)
        emit_a_tail(0)

        # ---- phase B (+ interleaved next-head A / phase C) ----
        sA = float(0.125 * np.log2(np.e) * 128.0)
        sB = float(127.0 * 128.0 - 5.58 + 0.5)
        EG = 2
        for j in range(HPC):
            for qi in range(NQC):
                qs = slice(qi * QC, (qi + 1) * QC)
                poa = ppO.tile([128, QC], F32, tag="oa", name="poa")
                for g in range(NKB // EG):
                    pss = ppS.tile([128, EG, QC], F32, tag="s", name="pss")
                    for i in range(EG):
                        kb = EG * g + i
                        r = slice(64 * (kb & 1), 64 * (kb & 1) + 64)
                        lhs = KQT[j] if (kb & 1) == 0 else QKT[j]
                        rhs = QKT[j] if (kb & 1) == 0 else KQT[j]
                        nc.tensor.matmul(
                            pss[:, i, :],
                            lhsT=lhs[r, kb * 128:(kb + 1) * 128],
                            rhs=rhs[r, qs],
                            start=True, stop=True)
                    es = epool.tile([128, EG, QC], BF16, tag="e", name="es")
                    if g in DVE_EXP_GROUPS:
                        nc.vector.tensor_scalar(
                            out=es.bitcast(I16), in0=pss,
                            scalar1=sA, scalar2=sB,
                            op0=mybir.AluOpType.mult,
                            op1=mybir.AluOpType.add)
                    else:
                        nc.scalar.activation(
                            out=es, in_=pss,
                            func=mybir.ActivationFunctionType.Exp,
                            scale=0.125)
                    for i in range(EG):
                        kb = EG * g + i
                        nc.tensor.matmul(
                            poa[0:DK + 1, :],
                            lhsT=V[j][:, kb, :], rhs=es[:, i, :],
                            start=(g == 0 and i == 0),
                            stop=(g == NKB // EG - 1 and i == EG - 1),
                            skip_group_check=True)
                # normalize: recip of sums -> broadcast -> scaled evacuation
                rr = rpool.tile([DK + 1, QC], F32, tag="rr", name="rr")
                nc.vector.reciprocal(
                    out=rr[DK:DK + 1, :], in_=poa[DK:DK + 1, :])
                rr0 = rpool.tile([1, QC], F32, tag="rr0", name="rr0")
                nc.sync.dma_start(out=rr0, in_=rr[DK:DK + 1, :])
                rbc = rpool.tile([DK, QC], F32, tag="rbc", name="rbc")
                nc.gpsimd.partition_broadcast(rbc, rr0, channels=DK)
                if j == 1:
                    ot1 = rpool.tile([DK, QC], F16, tag="ot1", name="ot1")
                    nc.vector.tensor_mul(
                        out=ot1, in0=poa[0:DK, :], in1=rbc)
                    nc.sync.dma_start(out=OT01[64:128, qs], in_=ot1)
                else:
                    tgt = OT2 if j == 2 else OT01
                    nc.vector.tensor_mul(
                        out=tgt[0:DK, qs], in0=poa[0:DK, :], in1=rbc)
                if j < 2:
                    # interleave next head's projections under this head's
                    # attention stream
                    emit_a_chunk(j + 1, qi)
                    if qi == NQC - 1:
                        emit_a_tail(j + 1)
                else:
                    # phase C one q-chunk behind so its matmuls never wait
                    # on this qi's normalization chain
                    if qi > 0:
                        emit_c(qi - 1)
                    if qi == NQC - 1:
                        emit_c(qi)


def shard_inputs_v3(x, Wq, bq, Wk, bk, Wv, bv, Wo, bo):
    x = np.asarray(x, np.float32)
    Wq, Wk, Wv = (np.asarray(a, np.float32) for a in (Wq, Wk, Wv))
    bq, bk, bv = (np.asarray(a, np.float32) for a in (bq, bk, bv))
    Wo = np.asarray(Wo, np.float32)
    in_maps = []
    for c in range(N_CORES):
        b, grp = divmod(c, 4)
        heads = [3 * grp + j for j in range(HPC)]
        wp = np.zeros((HPC, 2, 6, 128, 128), np.float16)
        bp = np.zeros((128, HPC, 2), np.float32)
        for j, h in enumerate(heads):
            sl = slice(64 * h, 64 * h + 64)
            wp[j, 0, :, :, 0:64] = Wq[sl].T.reshape(6, 128, 64)
            wp[j, 0, :, :, 64:128] = Wk[sl].T.reshape(6, 128, 64)
            bp[0:64, j, 0] = bq[sl]
            bp[64:128, j, 0] = bk[sl]
        sl0, sl1, sl2 = (slice(64 * h, 64 * h + 64) for h in heads)
        # V projections packed ahead: head0 chunk -> V0|V1, head1 -> V2
        wp[0, 1, :, :, 0:64] = Wv[sl0].T.reshape(6, 128, 64)
        wp[0, 1, :, :, 64:128] = Wv[sl1].T.reshape(6, 128, 64)
        wp[1, 1, :, :, 0:64] = Wv[sl2].T.reshape(6, 128, 64)
        bp[0:64, 0, 1] = bv[sl0]
        bp[64:128, 0, 1] = bv[sl1]
        bp[0:64, 1, 1] = bv[sl2]
        wo01 = np.concatenate([Wo[:, sl0].T, Wo[:, sl1].T], axis=0)
        in_maps.append({
            "xT": np.ascontiguousarray(x[b].T).astype(np.float16),
            "wp": wp, "bp": bp,
            "wo01": wo01.astype(np.float16),
            "wo2": Wo[:, sl2].T.astype(np.float16),
        })
    return in_maps


def build_program(debug=False, repeat=1, mode=None):
    mode = mode or KERNEL_MODE
    if mode == "v3":
        assert not debug
        return build_program_v3(repeat=repeat)
    nc = bacc.Bacc("TRN2", debug=False, num_devices=N_CORES)

    xT_d = nc.dram_tensor("xT", [D, S], F32R, kind="ExternalInput").ap()
    if mode.startswith("v2"):
        wp_d = nc.dram_tensor("wp", [HPC, 2, 6, 128, 128], F32R,
                              kind="ExternalInput").ap()
        bp_d = nc.dram_tensor("bp", [128, HPC, 2], F32,
                              kind="ExternalInput").ap()
    else:
        wp_d = nc.dram_tensor("wp", [5, 6, 128, 128], F32R,
                              kind="ExternalInput").ap()
        bp_d = nc.dram_tensor("bp", [128, 5], F32, kind="ExternalInput").ap()
    wo_d = nc.dram_tensor("wo", [HPC, DK, D], F32R, kind="ExternalInput").ap()
    out_d = nc.dram_tensor("out", [S, D], F32, kind="ExternalOutput").ap()

    dbg = {}
    if debug:
        dbg["qt"] = nc.dram_tensor("d_qt", [128, S], BF16,
                                   kind="ExternalOutput").ap()
        dbg["kt"] = nc.dram_tensor("d_kt", [128, S], BF16,
                                   kind="ExternalOutput").ap()
        dbg["v"] = nc.dram_tensor("d_v", [128, NKB, DK + 1], BF16,
                                  kind="ExternalOutput").ap()
        dbg["es"] = nc.dram_tensor("d_es", [128, 2, QC], BF16,
                                   kind="ExternalOutput").ap()
        dbg["po"] = nc.dram_tensor("d_po", [2, DK + 1, QC], F32,
                                   kind="ExternalOutput").ap()
        dbg["otr"] = nc.dram_tensor("d_otr", [DK + 1, QC], F32,
                                    kind="ExternalOutput").ap()
        dbg["rbc"] = nc.dram_tensor("d_rbc", [DK + 1, QC], F32,
                                    kind="ExternalOutput").ap()
        dbg["ot"] = nc.dram_tensor("d_ot", [DK + 1, S], F32,
                                   kind="ExternalOutput").ap()

    with tile.TileContext(nc) as tc, \
            nc.allow_low_precision("bf16/fp32r attention pipeline"):
        if mode.startswith("v2"):
            assert not debug and repeat >= 1
            for _ in range(repeat):
                _emit_v2(nc, tc, xT_d, wp_d, bp_d, wo_d, out_d,
                         exp_group=4 if mode == "v2_e4" else 2)
        else:
            _emit(nc, tc, xT_d, wp_d, bp_d, wo_d, out_d, dbg,
                  repeat=repeat, mode=mode)
    nc.compile()
    return nc


def _emit(nc, tc, xT_d, wp_d, bp_d, wo_d, out_d, dbg={},
          repeat=1, mode="tiled64"):
    import contextlib
    ctx = contextlib.ExitStack()
    with ctx:
        wpool = ctx.enter_context(tc.tile_pool(name="wpool", bufs=1))
        persist = ctx.enter_context(tc.tile_pool(name="persist", bufs=1))
        xpool = ctx.enter_context(tc.tile_pool(name="xpool", bufs=2))
        epool = ctx.enter_context(tc.tile_pool(name="epool", bufs=3))
        rpool = ctx.enter_context(tc.tile_pool(name="rpool", bufs=1))
        opool = ctx.enter_context(tc.tile_pool(name="opool", bufs=2))
        ppS = ctx.enter_context(tc.tile_pool(name="ppS", bufs=2, space="PSUM"))
        ppO = ctx.enter_context(tc.tile_pool(name="ppO", bufs=1, space="PSUM"))
        ppA = ctx.enter_context(tc.tile_pool(name="ppA", bufs=2, space="PSUM"))

        # ---- constants / weights ----
        wsb = wpool.tile([128, 5, 6, 128], F32R)
        nc.sync.dma_start(out=wsb, in_=wp_d.rearrange("g c p m -> p g c m"))
        bsb = wpool.tile([128, 5], F32)
        nc.sync.dma_start(out=bsb, in_=bp_d)
        wosb = wpool.tile([DK, HPC, D], F32R)
        nc.sync.dma_start(out=wosb, in_=wo_d.rearrange("j d m -> d j m"))
        ident = wpool.tile([128, 128], BF16)
        make_identity(nc, ident)

        assert not (dbg and repeat > 1)
        # which half each (head, kind) tensor is written to by the packed
        # projections, derived from PROJ_GROUPS
        wr_half = {}
        for gi, (mA, mB) in enumerate(PROJ_GROUPS):
            if gi == 4:
                wr_half[mA] = 0  # written to both halves
                continue
            wr_half[mA] = 0
            wr_half[mB] = 1

        for rep in range(repeat):
            # ---- persistent per-head tensors ----
            # QT/KT: [head_dim(64) in both halves (tiled64) or lower half +
            # zero upper (pad128), seq] bf16
            QT = [persist.tile([128, S], BF16, tag=f"qt{j}", name=f"qt{j}")
                  for j in range(HPC)]
            KT = [persist.tile([128, S], BF16, tag=f"kt{j}", name=f"kt{j}")
                  for j in range(HPC)]
            # V natural layout + ones column: [128 part = k%128, kb, 65]
            V = [persist.tile([128, NKB, DK + 1], BF16, tag=f"v{j}",
                              name=f"v{j}") for j in range(HPC)]
            # O^T staging: rows 0..63 = head dims, row 64 = softmax sums
            OT = [persist.tile([DK + 1, S], OT_DT, tag=f"ot{j}",
                               name=f"ot{j}") for j in range(HPC)]
            # VT transient [dims(64) at written half, seq] bf16
            VT = [persist.tile([128, S], BF16, tag=f"vt{j}", name=f"vt{j}")
                  for j in range(HPC)]

            for j in range(HPC):
                nc.vector.memset(V[j][:, :, DK], 1.0)

            def tgt(j, kind):
                return QT[j] if kind == 0 else KT[j] if kind == 1 else VT[j]

            # ---- Phase A: projections, x streamed in contraction-complete
            # column chunks ----
            n_xch = S // XCH
            for ci in range(n_xch):
                xq = xpool.tile([128, 6, XCH], F32R, tag="x", name="xq")
                nc.sync.dma_start(
                    out=xq,
                    in_=xT_d.rearrange("(c p) q -> p c q", p=128)[
                        :, :, ci * XCH:(ci + 1) * XCH],
                )
                for gi, (mA, mB) in enumerate(PROJ_GROUPS):
                    ps = ppA.tile([128, XCH], F32, tag="s", name="ps")
                    for c in range(6):
                        nc.tensor.matmul(
                            ps, lhsT=wsb[:, gi, c, :], rhs=xq[:, c, :],
                            start=(c == 0), stop=(c == 5))
                    # evacuate halves with bias add, cast to bf16
                    if gi == 4:
                        # V2 written to both halves at once (dup'd weights)
                        nc.vector.tensor_scalar_add(
                            out=VT[2][:, ci * XCH:(ci + 1) * XCH],
                            in0=ps, scalar1=bsb[:, gi:gi + 1])
                        continue
                    for half, (j, kind) in ((0, mA), (1, mB)):
                        lo, hi = half * 64, half * 64 + 64
                        nc.vector.tensor_scalar_add(
                            out=tgt(j, kind)[lo:hi, ci * XCH:(ci + 1) * XCH],
                            in0=ps[lo:hi, :],
                            scalar1=bsb[lo:hi, gi:gi + 1])

            # fix up Q/K halves (V^T needs none: transposes read the
            # written half directly)
            for j in range(HPC):
                for kind in (0, 1):
                    t = tgt(j, kind)
                    wh = wr_half[(j, kind)]
                    lo, hi = wh * 64, wh * 64 + 64
                    olo, ohi = 64 - lo, 128 - lo
                    if mode == "tiled64":
                        # duplicate into the other half
                        nc.sync.dma_start(out=t[olo:ohi, :], in_=t[lo:hi, :])
                    else:
                        # data to lower half, zero upper
                        if wh == 1:
                            nc.sync.dma_start(out=t[0:64, :], in_=t[64:128, :])
                        nc.vector.memset(t[64:128, :], 0.0)

            # V: transpose VT [dims, seq] -> natural [seq, dims] per block
            for j in range(HPC):
                voff = wr_half[(j, 2)] * 64
                for kb in range(NKB):
                    pt = ppA.tile([128, 128], BF16, tag="s", name="pt")
                    nc.tensor.transpose(
                        pt, VT[j][:, kb * 128:(kb + 1) * 128], ident)
                    nc.vector.tensor_copy(
                        out=V[j][:, kb, 0:DK], in_=pt[:, voff:voff + DK])

            if dbg:
                nc.sync.dma_start(out=dbg["qt"], in_=QT[0])
                nc.sync.dma_start(out=dbg["kt"], in_=KT[0])
                nc.sync.dma_start(out=dbg["v"], in_=V[0])

            # ---- Phase B: attention per head ----
            for j in range(HPC):
                for qi in range(NQC):
                    qs = qi * QC
                    poa = ppO.tile([DK + 1, QC], F32, tag="oa", name="poa")
                    if mode == "tiled64":
                        pob = ppO.tile([DK + 1, QC], F32, tag="ob",
                                       name="pob")
                    for p in range(NKB // 2):  # pairs of k-blocks
                        pss = ppS.tile([128, 2, QC], F32, tag="s", name="pss")
                        if mode == "tiled64":
                            # two concurrent 64-contraction row tiles
                            nc.tensor.matmul(
                                pss[:, 0, :],
                                lhsT=KT[j][0:64, p * 256:p * 256 + 128],
                                rhs=QT[j][0:64, qs:qs + QC],
                                start=True, stop=True)
                            nc.tensor.matmul(
                                pss[:, 1, :],
                                lhsT=KT[j][64:128, p * 256 + 128:p * 256 + 256],
                                rhs=QT[j][64:128, qs:qs + QC],
                                start=True, stop=True)
                        else:
                            for s in range(2):
                                kb = 2 * p + s
                                nc.tensor.matmul(
                                    pss[:, s, :],
                                    lhsT=KT[j][:, kb * 128:(kb + 1) * 128],
                                    rhs=QT[j][:, qs:qs + QC],
                                    start=True, stop=True)
                        es = epool.tile([128, 2, QC], BF16, tag="e", name="es")
                        nc.scalar.activation(
                            out=es, in_=pss,
                            func=mybir.ActivationFunctionType.Exp, scale=0.125)
                        if dbg and j == 0 and qi == 0 and p == 0:
                            nc.sync.dma_start(out=dbg["es"], in_=es)
                        for s in range(2):
                            kb = 2 * p + s
                            first = p == 0 and s == 0
                            last = p == NKB // 2 - 1 and s == 1
                            if mode == "tiled64":
                                nc.tensor.matmul(
                                    poa, lhsT=V[j][0:64, kb, :],
                                    rhs=es[0:64, s, :],
                                    start=first, stop=last,
                                    skip_group_check=True)
                                nc.tensor.matmul(
                                    pob, lhsT=V[j][64:128, kb, :],
                                    rhs=es[64:128, s, :],
                                    start=first, stop=last,
                                    skip_group_check=True)
                            else:
                                nc.tensor.matmul(
                                    poa, lhsT=V[j][:, kb, :],
                                    rhs=es[:, s, :],
                                    start=first, stop=last,
                                    skip_group_check=True)
                    # evacuate (DVE may read only one PSUM operand per op)
                    nc.vector.tensor_copy(out=OT[j][:, qs:qs + QC], in_=poa)
                    if mode == "tiled64":
                        nc.vector.tensor_add(
                            out=OT[j][:, qs:qs + QC],
                            in0=OT[j][:, qs:qs + QC], in1=pob)
                    # reciprocal of sums in place (row 64)
                    nc.vector.reciprocal(
                        out=OT[j][DK:DK + 1, qs:qs + QC],
                        in_=OT[j][DK:DK + 1, qs:qs + QC])
                    if dbg and j == 0 and qi == 0:
                        nc.sync.dma_start(
                            out=dbg["otr"],
                            in_=OT[0][:, 0:QC].bitcast(F32))
                    # broadcast recip across partitions and scale O^T.
                    # partition_broadcast reads PHYSICAL partition 0, so
                    # stage the recip row there via a tiny DMA first.
                    srow = rpool.tile([1, QC], OT_DT, tag="sr", name="srow")
                    nc.sync.dma_start(
                        out=srow, in_=OT[j][DK:DK + 1, qs:qs + QC])
                    rbc = rpool.tile([DK + 1, QC], OT_DT, tag="r", name="rbc")
                    nc.gpsimd.partition_broadcast(rbc, srow, channels=DK + 1)
                    if dbg and j == 0 and qi == 0:
                        nc.sync.dma_start(out=dbg["rbc"], in_=rbc.bitcast(F32))
                    nc.vector.tensor_mul(
                        out=OT[j][0:DK, qs:qs + QC],
                        in0=OT[j][0:DK, qs:qs + QC], in1=rbc[0:DK, :])

            if dbg:
                nc.sync.dma_start(out=dbg["ot"], in_=OT[0].bitcast(F32))

            # ---- Phase C: output projection ----
            for t in range(S // 128):
                c1 = ppA.tile([128, 512], F32, tag="s", name="c1")
                c2 = ppA.tile([128, 256], F32, tag="s", name="c2")
                for j in range(HPC):
                    nc.tensor.matmul(
                        c1, lhsT=OT[j][0:DK, t * 128:(t + 1) * 128],
                        rhs=wosb[:, j, 0:512],
                        start=(j == 0), stop=(j == HPC - 1))
                for j in range(HPC):
                    nc.tensor.matmul(
                        c2, lhsT=OT[j][0:DK, t * 128:(t + 1) * 128],
                        rhs=wosb[:, j, 512:768],
                        start=(j == 0), stop=(j == HPC - 1))
                ot = opool.tile([128, D], F32, tag="o", name="ot")
                nc.vector.tensor_copy(out=ot[:, 0:512], in_=c1)
                nc.vector.tensor_copy(out=ot[:, 512:768], in_=c2)
                nc.sync.dma_start(out=out_d[t * 128:(t + 1) * 128, :], in_=ot)




def _emit_v2(nc, tc, xT_d, wp_d, bp_d, wo_d, out_d, exp_group=4):
    """Per-head pipeline; S^T psum in bf16 when exp_group=4 (2048-wide exp)."""
    import contextlib
    ctx = contextlib.ExitStack()
    with ctx:
        wpool = ctx.enter_context(tc.tile_pool(name="wpool", bufs=1))
        persist = ctx.enter_context(tc.tile_pool(name="persist", bufs=1))
        hpool = ctx.enter_context(tc.tile_pool(name="hpool", bufs=2))
        xpool = ctx.enter_context(tc.tile_pool(name="xpool", bufs=2))
        epool = ctx.enter_context(tc.tile_pool(name="epool", bufs=4))
        rpool = ctx.enter_context(tc.tile_pool(name="rpool", bufs=1))
        opool = ctx.enter_context(tc.tile_pool(name="opool", bufs=2))
        # one shared PSUM pool for S-groups/proj/transposes/phase C
        # (3 slots of 2 banks) + the two O accumulators (1 bank each)
        ppS = ctx.enter_context(tc.tile_pool(name="ppS", bufs=3, space="PSUM"))
        ppO = ctx.enter_context(tc.tile_pool(name="ppO", bufs=1, space="PSUM"))
        ppA = ppS

        SDT = BF16 if exp_group == 4 else F32
        NG = NKB // exp_group

        wsb = wpool.tile([128, HPC, 2, 6, 128], F32R)
        nc.sync.dma_start(out=wsb, in_=wp_d.rearrange("j g c p m -> p j g c m"))
        bsb = wpool.tile([128, HPC, 2], F32)
        nc.sync.dma_start(out=bsb, in_=bp_d)
        wosb = wpool.tile([DK, HPC, D], F32R)
        nc.sync.dma_start(out=wosb, in_=wo_d.rearrange("j d m -> d j m"))
        ident = wpool.tile([128, 128], BF16)
        make_identity(nc, ident)

        OT = [persist.tile([DK + 1, S], OT_DT, tag=f"ot{j}", name=f"ot{j}")
              for j in range(HPC)]

        def emit_c(cqi):
            for t in range(cqi * QC // 128, (cqi + 1) * QC // 128):
                c1 = ppO.tile([128, 512], F32, tag="oa", name="c1")
                c2 = ppO.tile([128, 256], F32, tag="ob", name="c2")
                for jj in range(HPC):
                    nc.tensor.matmul(
                        c1, lhsT=OT[jj][0:DK, t * 128:(t + 1) * 128],
                        rhs=wosb[:, jj, 0:512],
                        start=(jj == 0), stop=(jj == HPC - 1))
                for jj in range(HPC):
                    nc.tensor.matmul(
                        c2, lhsT=OT[jj][0:DK, t * 128:(t + 1) * 128],
                        rhs=wosb[:, jj, 512:768],
                        start=(jj == 0), stop=(jj == HPC - 1))
                ot = opool.tile([128, D], F32, tag="o", name="ot")
                nc.vector.tensor_copy(out=ot[:, 0:512], in_=c1)
                nc.vector.tensor_copy(out=ot[:, 512:768], in_=c2)
                nc.sync.dma_start(
                    out=out_d[t * 128:(t + 1) * 128, :], in_=ot)

        n_xch = S // XCH
        for j in range(HPC):
            # ---- phase A for head j ----
            QT = hpool.tile([128, S], BF16, tag="qt", name="qt")
            KT = hpool.tile([128, S], BF16, tag="kt", name="kt")
            VT = hpool.tile([128, S], BF16, tag="vt", name="vt")
            V = hpool.tile([128, NKB, DK + 1], BF16, tag="v", name="v")
            nc.vector.memset(V[:, :, DK], 1.0)
            for ci in range(n_xch):
                xq = xpool.tile([128, 6, XCH], F32R, tag="x", name="xq")
                nc.sync.dma_start(
                    out=xq,
                    in_=xT_d.rearrange("(c p) q -> p c q", p=128)[
                        :, :, ci * XCH:(ci + 1) * XCH])
                cs = slice(ci * XCH, (ci + 1) * XCH)
                # group 0: (Q | K)
                ps = ppA.tile([128, XCH], F32, tag="s", name="ps")
                for c in range(6):
                    nc.tensor.matmul(
                        ps, lhsT=wsb[:, j, 0, c, :], rhs=xq[:, c, :],
                        start=(c == 0), stop=(c == 5))
                nc.vector.tensor_scalar_add(
                    out=QT[0:64, cs], in0=ps[0:64, :],
                    scalar1=bsb[0:64, j, 0:1])
                nc.vector.tensor_scalar_add(
                    out=KT[64:128, cs], in0=ps[64:128, :],
                    scalar1=bsb[64:128, j, 0:1])
                # group 1: (V | V) duplicated
                ps2 = ppA.tile([128, XCH], F32, tag="s", name="ps2")
                for c in range(6):
                    nc.tensor.matmul(
                        ps2, lhsT=wsb[:, j, 1, c, :], rhs=xq[:, c, :],
                        start=(c == 0), stop=(c == 5))
                nc.vector.tensor_scalar_add(
                    out=VT[:, cs], in0=ps2, scalar1=bsb[:, j, 1:2])
                # V natural layout via PE transposes (chunk's k-blocks)
                for kb in range(ci * XCH // 128, (ci + 1) * XCH // 128):
                    pt = ppA.tile([128, 128], BF16, tag="s", name="pt")
                    nc.tensor.transpose(
                        pt, VT[:, kb * 128:(kb + 1) * 128], ident)
                    nc.vector.tensor_copy(
                        out=V[:, kb, 0:DK], in_=pt[:, 0:DK])
            # duplicate halves: Q lower->upper, K upper->lower
            nc.sync.dma_start(out=QT[64:128, :], in_=QT[0:64, :])
            nc.sync.dma_start(out=KT[0:64, :], in_=KT[64:128, :])

            # ---- phase B for head j ----
            for qi in range(NQC):
                qs = qi * QC
                poa = ppO.tile([DK + 1, QC], F32, tag="oa", name="poa")
                pob = ppO.tile([DK + 1, QC], F32, tag="ob", name="pob")
                for g in range(NG):
                    pss = ppS.tile([128, exp_group, QC], SDT, tag="s",
                                   name="pss")
                    # T0 row-tile: first half of the group's k-blocks;
                    # T8: second half (separate PSUM banks)
                    hg = exp_group // 2
                    for i in range(hg):
                        kb = g * exp_group + i
                        nc.tensor.matmul(
                            pss[:, i, :],
                            lhsT=KT[0:64, kb * 128:(kb + 1) * 128],
                            rhs=QT[0:64, qs:qs + QC],
                            start=True, stop=True)
                    for i in range(hg):
                        kb = g * exp_group + hg + i
                        nc.tensor.matmul(
                            pss[:, hg + i, :],
                            lhsT=KT[64:128, kb * 128:(kb + 1) * 128],
                            rhs=QT[64:128, qs:qs + QC],
                            start=True, stop=True)
                    es = epool.tile([128, exp_group, QC], BF16, tag="e",
                                    name="es")
                    nc.scalar.activation(
                        out=es, in_=pss,
                        func=mybir.ActivationFunctionType.Exp, scale=0.125)
                    for s in range(exp_group):
                        kb = g * exp_group + s
                        first = g == 0 and s == 0
                        last = g == NG - 1 and s == exp_group - 1
                        nc.tensor.matmul(
                            poa, lhsT=V[0:64, kb, :], rhs=es[0:64, s, :],
                            start=first, stop=last, skip_group_check=True)
                        nc.tensor.matmul(
                            pob, lhsT=V[64:128, kb, :], rhs=es[64:128, s, :],
                            start=first, stop=last, skip_group_check=True)
                nc.vector.tensor_copy(out=OT[j][:, qs:qs + QC], in_=poa)
                nc.vector.tensor_add(
                    out=OT[j][:, qs:qs + QC],
                    in0=OT[j][:, qs:qs + QC], in1=pob)
                nc.vector.reciprocal(
                    out=OT[j][DK:DK + 1, qs:qs + QC],
                    in_=OT[j][DK:DK + 1, qs:qs + QC])
                srow = rpool.tile([1, QC], OT_DT, tag="sr", name="srow")
                nc.sync.dma_start(
                    out=srow, in_=OT[j][DK:DK + 1, qs:qs + QC])
                rbc = rpool.tile([DK + 1, QC], OT_DT, tag="r", name="rbc")
                nc.gpsimd.partition_broadcast(rbc, srow, channels=DK + 1)
                nc.vector.tensor_mul(
                    out=OT[j][0:DK, qs:qs + QC],
                    in0=OT[j][0:DK, qs:qs + QC], in1=rbc[0:DK, :])

        # ---- phase C: output projection (borrows psumO slots) ----
        for cqi in range(NQC):
            emit_c(cqi)




# ---------------------------------------------------------------------------
# host side
# ---------------------------------------------------------------------------

KERNEL_MODE = "v3"


def shard_inputs(x, Wq, bq, Wk, bk, Wv, bv, Wo, bo, mode=None):
    """Build the 8 per-core input maps."""
    mode = mode or KERNEL_MODE
    if mode == "v3":
        return shard_inputs_v3(x, Wq, bq, Wk, bk, Wv, bv, Wo, bo)
    if mode.startswith("v2"):
        return shard_inputs_v2(x, Wq, bq, Wk, bk, Wv, bv, Wo, bo)
    return shard_inputs_v1(x, Wq, bq, Wk, bk, Wv, bv, Wo, bo)


def shard_inputs_v2(x, Wq, bq, Wk, bk, Wv, bv, Wo, bo):
    x = np.asarray(x, np.float32)
    Wq, Wk, Wv = (np.asarray(a, np.float32) for a in (Wq, Wk, Wv))
    bq, bk, bv = (np.asarray(a, np.float32) for a in (bq, bk, bv))
    Wo = np.asarray(Wo, np.float32)
    in_maps = []
    for c in range(N_CORES):
        b, g = divmod(c, 4)
        heads = [3 * g + j for j in range(HPC)]
        wp = np.empty((HPC, 2, 6, 128, 128), np.float32)
        bp = np.zeros((128, HPC, 2), np.float32)
        wo = np.empty((HPC, DK, D), np.float32)
        for j, h in enumerate(heads):
            sl = slice(64 * h, 64 * h + 64)
            wp[j, 0, :, :, 0:64] = Wq[sl].T.reshape(6, 128, 64)
            wp[j, 0, :, :, 64:128] = Wk[sl].T.reshape(6, 128, 64)
            wp[j, 1, :, :, 0:64] = Wv[sl].T.reshape(6, 128, 64)
            wp[j, 1, :, :, 64:128] = Wv[sl].T.reshape(6, 128, 64)
            bp[0:64, j, 0] = bq[sl]
            bp[64:128, j, 0] = bk[sl]
            bp[0:64, j, 1] = bv[sl]
            bp[64:128, j, 1] = bv[sl]
            wo[j] = Wo[:, sl].T
        in_maps.append({
            "xT": np.ascontiguousarray(x[b].T),
            "wp": wp, "bp": bp, "wo": wo,
        })
    return in_maps


def shard_inputs_v1(x, Wq, bq, Wk, bk, Wv, bv, Wo, bo):
    """Build the 8 per-core input maps."""
    x = np.asarray(x, np.float32)
    Ws = {0: np.asarray(Wq, np.float32), 1: np.asarray(Wk, np.float32),
          2: np.asarray(Wv, np.float32)}
    bs = {0: np.asarray(bq, np.float32), 1: np.asarray(bk, np.float32),
          2: np.asarray(bv, np.float32)}
    Wo = np.asarray(Wo, np.float32)
    in_maps = []
    for c in range(N_CORES):
        b, g = divmod(c, 4)
        heads = [3 * g + j for j in range(HPC)]
        wp = np.empty((5, 6, 128, 128), np.float32)
        bp = np.zeros((128, 5), np.float32)
        for gi, (mA, mB) in enumerate(PROJ_GROUPS):
            for half, (j, kind) in ((0, mA), (1, mB)):
                h = heads[j]
                Wh = Ws[kind][64 * h:64 * h + 64, :]       # [64, 768]
                chunks = Wh.T.reshape(6, 128, 64)          # [c, p, 64]
                wp[gi, :, :, half * 64:half * 64 + 64] = chunks
                bp[half * 64:half * 64 + 64, gi] = bs[kind][64 * h:64 * h + 64]
        wo = np.empty((HPC, DK, D), np.float32)
        for j in range(HPC):
            h = heads[j]
            wo[j] = Wo[:, 64 * h:64 * h + 64].T
        in_maps.append({
            "xT": np.ascontiguousarray(x[b].T),
            "wp": wp, "bp": bp, "wo": wo,
        })
    return in_maps


def assemble_output(parts, bo):
    out = np.empty((B, S, D), np.float32)
    for b in range(B):
        acc = parts[4 * b]["out"].astype(np.float32).copy()
        for c in range(4 * b + 1, 4 * b + 4):
            acc += parts[c]["out"]
        out[b] = acc + np.asarray(bo, np.float32)[None, :]
    return out


_RUNNER = None


def _make_runner(nc):
    """Reusable PJRT runner (mirrors bass2jax.run_bass_via_pjrt multi-core)."""
    import jax
    import jax.numpy as jnp
    from jax.experimental.shard_map import shard_map
    from jax.sharding import Mesh, PartitionSpec
    from concourse import bass2jax

    bass2jax.install_neuronx_cc_hook()

    partition_name = (nc.partition_id_tensor.name
                      if nc.partition_id_tensor else None)
    in_names, out_names, out_avals = [], [], []
    for alloc in nc.m.functions[0].allocations:
        if not isinstance(alloc, mybir.MemoryLocationSet):
            continue
        name = alloc.memorylocations[0].name
        if alloc.kind == "ExternalInput":
            if name != partition_name:
                in_names.append(name)
        elif alloc.kind == "ExternalOutput":
            out_names.append(name)
            out_avals.append(jax.core.ShapedArray(
                tuple(alloc.tensor_shape), mybir.dt.np(alloc.dtype)))
    n_params = len(in_names)
    n_outs = len(out_names)
    all_in_names = list(in_names) + list(out_names)
    if partition_name is not None:
        all_in_names.append(partition_name)
    donate = tuple(range(n_params, n_params + n_outs))

    def _body(*args):
        operands = list(args)
        if partition_name is not None:
            operands.append(bass2jax.partition_id_tensor())
        outs = bass2jax._bass_exec_p.bind(
            *operands,
            out_avals=tuple(out_avals),
            in_names=tuple(all_in_names),
            out_names=tuple(out_names),
            lowering_input_output_aliases=(),
            sim_require_finite=True,
            sim_require_nnan=True,
            nc=nc,
        )
        return tuple(outs)

    devices = jax.devices()[:N_CORES]
    mesh = Mesh(np.asarray(devices), ("core",))
    in_specs = (PartitionSpec("core"),) * (n_params + n_outs)
    out_specs = (PartitionSpec("core"),) * n_outs
    sharded = jax.jit(
        shard_map(_body, mesh=mesh, in_specs=in_specs, out_specs=out_specs,
                  check_rep=False),
        donate_argnums=donate, keep_unused=True)

    def run(in_maps):
        per_core = [[np.asarray(m[name]) for name in in_names]
                    for m in in_maps]
        concat_in = [np.concatenate([per_core[c][i] for c in range(N_CORES)],
                                    axis=0) for i in range(n_params)]
        zeros = [np.zeros((N_CORES * av.shape[0], *av.shape[1:]), av.dtype)
                 for av in out_avals]
        outs = sharded(*concat_in, *zeros)
        return [
            {name: np.asarray(outs[i]).reshape(N_CORES, *out_avals[i].shape)[c]
             for i, name in enumerate(out_names)}
            for c in range(N_CORES)
        ]

    run.sharded = sharded
    run.in_names = in_names
    run.out_names = out_names
    run.out_avals = out_avals
    run.n_params = n_params
    return run


def get_runner():
    global _RUNNER
    if _RUNNER is None:
        nc = build_program()
        _RUNNER = _make_runner(nc)
    return _RUNNER


def kernel(x, Wq, bq, Wk, bk, Wv, bv, Wo, bo):
    run = get_runner()
    in_maps = shard_inputs(x, Wq, bq, Wk, bk, Wv, bv, Wo, bo)
    parts = run(in_maps)
    return assemble_output(parts, bo)

